# revision 1
# baseline (speedup 1.0000x reference)
"""BFP-quantized GEMM (nn_CustomLinear) on 8 trn2 NeuronCores.

out = bfp_quant(x) @ bfp_quant(weight).T + bias
  x [4096,4096] f32, weight [4096,4096] f32, bias [4096] f32
  BFP: groups of 16 along K share exponent floor(log2(max|x|)); 8-bit
  signed mantissa; dequantized values are exactly representable in bf16
  (<=8 significant bits times a power of two), so the matmul runs on the
  PE in bf16 with exact products.

Sharding: column-parallel. weight/bias sharded on N across 8 cores
(512 rows each), x replicated; per-core output [4096, 512], concatenated
on the host.

Quantization per tile [128, K] (groups along the free dim):
  maxabs = reduce_absmax over groups of 16
  ebits  = maxabs & 0x7f800000          (exponent field, e = unbiased)
  c      = bitcast(ebits + 0x08c00000)  = 1.5 * 2^(e+17) = 3*2^16 * s
  s      = bitcast(ebits - 0x03000000)  = 2^(e-6)
  d      = 127*s + c
  t1 = x + c      -> rounds x to a multiple of s, round-half-even,
                     exactly matching jnp.round(x/s) (x+c keeps exponent
                     e+17, so ulp == s throughout)
  t2 = min(t1, d) -> clips round(x/s) to <= 127 (>= -128 is automatic)
  xq = t2 - c     -> m*s with |m|<=127, exact; emitted as bf16 (exact)
"""

import sys

if "/opt/trn_rl_repo" not in sys.path:
    sys.path.insert(0, "/opt/trn_rl_repo")

import numpy as np

M, K, N = 4096, 4096, 4096
NCORES = 8
NSH = N // NCORES  # 512
P = 128
GROUP = 16
GK = K // GROUP  # 256 groups per row
KB = K // P      # 32 k-blocks
MT = M // P      # 32 m-tiles
NT = NSH // P    # 4 weight tiles per core

_EXP_MASK = 0x7F800000
_C_OFF = 0x08C00000   # +17 in exponent, 0x400000 mantissa -> *1.5
_S_OFF = 0x03000000   # -6 in exponent


_PATCHED = False


def _patch_multiwait_split():
    """Walrus in this container rejects >1 sync wait on DMA/engine
    instructions ("Too many sync wait commands"). After Tile's wait
    assignment, hoist excess waits onto standalone InstNoOp carriers on the
    same engine, immediately before the instruction (same-engine program
    order preserves the sync semantics)."""
    global _PATCHED
    if _PATCHED:
        return
    import concourse.tile as tile
    from concourse import mybir

    real = tile.TileClockWait

    class SplitWaits:
        def __init__(self, tc, blocks, **kw):
            self._inner = real(tc, blocks, **kw)
            self._blocks = blocks
            self._nc = tc.nc

        def assign_waits(self, *a, **kw):
            r = self._inner.assign_waits(*a, **kw)
            skip = (mybir.InstEventSemaphore,)
            for bb, insts in self._blocks.items():
                out = []
                for inst in insts:
                    si = inst.sync_info
                    if (
                        si is not None
                        and si.on_wait
                        and len(si.on_wait) > 1
                        and not isinstance(inst, skip)
                        and inst.engine != mybir.EngineType.Unassigned
                    ):
                        for w in si.on_wait[:-1]:
                            out.append(
                                mybir.InstNoOp(
                                    name=self._nc.get_next_instruction_name(),
                                    sync_info=mybir.SyncInfo(
                                        on_wait=[w], on_update=[]
                                    ),
                                    bass_nofuse=True,
                                    engine=inst.engine,
                                )
                            )
                        inst.sync_info = mybir.SyncInfo(
                            on_wait=[si.on_wait[-1]], on_update=si.on_update
                        )
                    out.append(inst)
                insts[:] = out
            return r

        def __getattr__(self, k):
            return getattr(self._inner, k)

    tile.TileClockWait = SplitWaits

    from concourse.vector_clock import ScopedClock

    def _drain_and_barrier(self, tick_clock, wait_clock):
        # Collect the tail waits on a nop, then fan the excess out onto
        # additional single-wait nops (SP executes them in order), and only
        # then drain + barrier. Mirrors TileContext._drain_and_barrier.
        tmp = self.nc.sync.nop(nofuse=True)
        wait_clock.add_sem_waits(
            tmp.ins, ScopedClock({None: tick_clock.global_clock})
        )
        si = tmp.ins.sync_info
        waits = list(si.on_wait) if si and si.on_wait else []
        if waits:
            tmp.ins.sync_info = mybir.SyncInfo(on_wait=[waits[0]], on_update=[])
            for w in waits[1:]:
                nxt = self.nc.sync.nop(nofuse=True)
                nxt.ins.sync_info = mybir.SyncInfo(on_wait=[w], on_update=[])
        self.nc.sync.drain()

        self.nc.all_engine_barrier()
        assert self.sems is not None
        popped = self.nc._tile_sem_poison_stack.pop()
        assert popped is self._sem_poison
        self.nc.clear_and_free_semaphores(list(self.sems.allocated().values()))
        self.nc.all_engine_barrier()

    tile.TileContext._drain_and_barrier = _drain_and_barrier
    _PATCHED = True


def _build_program(m=M, k=K, nsh=NSH, repeat=1):
    import concourse.bass as bass
    import concourse.tile as tile
    from concourse import mybir
    from concourse.masks import make_identity
    from contextlib import ExitStack

    _patch_multiwait_split()

    f32 = mybir.dt.float32
    bf16 = mybir.dt.bfloat16
    i32 = mybir.dt.int32

    GK = k // GROUP
    KB = k // P
    MT = m // P
    NT = nsh // P

    nc = bass.Bass()
    x_d = nc.dram_tensor("x", [m, k], f32, kind="ExternalInput")
    w_d = nc.dram_tensor("w", [nsh, k], f32, kind="ExternalInput")
    b_d = nc.dram_tensor("b", [nsh], f32, kind="ExternalInput")
    o_d = nc.dram_tensor("out", [m, nsh], f32, kind="ExternalOutput")

    def bcast16(t):
        # [P, GK] -> [P, GK, 16] with stride-0 inner dim
        return bass.AP(
            tensor=t.tensor,
            offset=t.offset,
            ap=[list(t.ap[0]), list(t.ap[1]), [0, GROUP]],
        )

    with ExitStack() as ctx:
        tc = ctx.enter_context(tile.TileContext(nc))

        const = ctx.enter_context(tc.tile_pool(name="const", bufs=1))
        ident = const.tile([P, P], bf16)
        make_identity(nc, ident)

        # Bias folded into the matmul as a K=2 rank-update: ones2.T @ brow
        # where brow = [bf16_hi(bias); bf16_lo(residual)] (exact to ~2^-18).
        ones1 = const.tile([1, P], bf16)
        nc.vector.memset(ones1, 1.0)
        bias_f = const.tile([1, nsh], f32)
        nc.gpsimd.dma_start(out=bias_f, in_=bass.AP(b_d, 0, [[0, 1], [1, nsh]]))
        b_hi = const.tile([1, nsh], bf16)
        nc.vector.tensor_copy(out=b_hi, in_=bias_f)
        blo_f = const.tile([1, nsh], f32)
        nc.vector.tensor_tensor(
            out=blo_f, in0=bias_f, in1=b_hi, op=mybir.AluOpType.subtract
        )
        b_lo = const.tile([1, nsh], bf16)
        nc.vector.tensor_copy(out=b_lo, in_=blo_f)

        # wqT[k % 128, kb, n] = quantized weight transposed, [K, NSH] as 32 blocks
        wqT = const.tile([P, KB, nsh], bf16)

        xt_pool = ctx.enter_context(tc.tile_pool(name="xt", bufs=3))
        t2_pool = ctx.enter_context(tc.tile_pool(name="t2", bufs=3))
        xq_pool = ctx.enter_context(tc.tile_pool(name="xq", bufs=3))
        xqT_pool = ctx.enter_context(tc.tile_pool(name="xqT", bufs=2))
        sm_pool = ctx.enter_context(tc.tile_pool(name="sm", bufs=3))
        cp_pool = ctx.enter_context(tc.tile_pool(name="cp", bufs=3))
        ob_pool = ctx.enter_context(tc.tile_pool(name="ob", bufs=3))
        tps_pool = ctx.enter_context(tc.tile_pool(name="tps", bufs=4, space="PSUM"))
        ops_pool = ctx.enter_context(tc.tile_pool(name="ops", bufs=2, space="PSUM"))

        def quantize(src_dram_rows):
            """DMA a [P, K] f32 row-tile in, return quantized bf16 [P, K] tile."""
            xt = xt_pool.tile([P, k], f32, tag="xt")
            nc.gpsimd.dma_start(out=xt, in_=src_dram_rows)

            xmax = sm_pool.tile([P, GK], f32, tag="xmax")
            nc.vector.tensor_reduce(
                out=xmax,
                in_=xt.rearrange("p (g j) -> p g j", j=GROUP),
                axis=mybir.AxisListType.X,
                op=mybir.AluOpType.max,
                apply_absolute_value=True,
            )
            eb_t = sm_pool.tile([P, GK], i32, tag="eb_t")
            nc.vector.tensor_scalar(
                out=eb_t,
                in0=xmax.bitcast(i32),
                scalar1=_EXP_MASK,
                scalar2=0,
                op0=mybir.AluOpType.bitwise_and,
                op1=mybir.AluOpType.bitwise_or,
            )
            c_t = sm_pool.tile([P, GK], f32, tag="c_t")
            nc.vector.tensor_scalar_add(
                out=c_t.bitcast(i32), in0=eb_t, scalar1=_C_OFF
            )
            s_t = sm_pool.tile([P, GK], f32, tag="s_t")
            nc.vector.tensor_scalar_sub(
                out=s_t.bitcast(i32), in0=eb_t, scalar1=_S_OFF
            )
            d_t = sm_pool.tile([P, GK], f32, tag="d_t")
            nc.vector.scalar_tensor_tensor(
                out=d_t,
                in0=s_t,
                scalar=127.0,
                in1=c_t,
                op0=mybir.AluOpType.mult,
                op1=mybir.AluOpType.add,
            )

            # In-place add (t1 = x + c), split between Pool and DVE to
            # balance engine load (Pool also runs the subtract + DMA
            # dispatch; it is ~2x slower per element and its ucode supports
            # add/subtract but not min). Pool reads ACT-made copies of the
            # small tensors so DVE smalls keep a single foreign reader.
            c_p = cp_pool.tile([P, GK], f32, tag="c_p")
            nc.scalar.copy(out=c_p, in_=c_t)
            asplit = (GK * 7) // 16  # Pool's share of the add
            xtv = xt.rearrange("p (g j) -> p g j", j=GROUP)
            nc.gpsimd.tensor_tensor(
                out=xtv[:, :asplit, :],
                in0=xtv[:, :asplit, :],
                in1=bcast16(c_p)[:, :asplit, :],
                op=mybir.AluOpType.add,
            )
            nc.vector.tensor_tensor(
                out=xtv[:, asplit:, :],
                in0=xtv[:, asplit:, :],
                in1=bcast16(c_t)[:, asplit:, :],
                op=mybir.AluOpType.add,
            )
            t2 = t2_pool.tile([P, k], f32, tag="t2")
            nc.vector.tensor_tensor(
                out=t2.rearrange("p (g j) -> p g j", j=GROUP),
                in0=xtv,
                in1=bcast16(d_t),
                op=mybir.AluOpType.min,
            )
            xq = xq_pool.tile([P, k], bf16, tag="xq")
            nc.gpsimd.tensor_tensor(
                out=xq.rearrange("p (g j) -> p g j", j=GROUP),
                in0=t2.rearrange("p (g j) -> p g j", j=GROUP),
                in1=bcast16(c_p),
                op=mybir.AluOpType.subtract,
            )
            return xq

        def transpose_to(xq, dest_slices):
            """PE-transpose [P, K] bf16 into dest_slices(kb) [P, P] blocks."""
            for j in range(KB // 4):
                pt = tps_pool.tile([P, 4, P], bf16, tag="tps")
                for i in range(4):
                    kb = 4 * j + i
                    nc.tensor.transpose(
                        pt[:, i, :], xq[:, kb * P : (kb + 1) * P], ident
                    )
                nc.scalar.copy(out=dest_slices(j), in_=pt)

        # ---- weight prep + main loop (repeat>1 only for benchmarking) ----
        for _rep in range(repeat):
            for nt in range(NT):
                wq = quantize(w_d[nt * P : (nt + 1) * P, :])
                transpose_to(
                    wq,
                    lambda j, nt=nt: wqT[:, 4 * j : 4 * j + 4, nt * P : (nt + 1) * P],
                )

            for mt in range(MT):
                xq = quantize(x_d[mt * P : (mt + 1) * P, :])
                xqT = xqT_pool.tile([P, KB, P], bf16, tag="xqT")
                transpose_to(xq, lambda j: xqT[:, 4 * j : 4 * j + 4, :])

                ps = ops_pool.tile([P, nsh], f32, tag="ops")
                for kb in range(KB):
                    nc.tensor.matmul(
                        ps,
                        xqT[:, kb, :],
                        wqT[:, kb, :],
                        start=(kb == 0),
                        stop=False,
                    )
                nc.tensor.matmul(ps, ones1, b_hi, start=False, stop=False)
                nc.tensor.matmul(ps, ones1, b_lo, start=False, stop=True)
                ob = ob_pool.tile([P, nsh], f32, tag="ob")
                nc.scalar.copy(out=ob, in_=ps)
                nc.gpsimd.dma_start(out=o_d[mt * P : (mt + 1) * P, :], in_=ob)

    nc.finalize()
    return nc


_NC = None


def _get_program():
    global _NC
    if _NC is None:
        _NC = _build_program()
    return _NC


def _run(x, weight, bias, **kw):
    from concourse.bass_utils import run_bass_kernel_spmd

    x = np.ascontiguousarray(x, dtype=np.float32)
    weight = np.ascontiguousarray(weight, dtype=np.float32)
    bias = np.ascontiguousarray(bias, dtype=np.float32)

    nc = _get_program()
    in_maps = [
        {
            "x": x,
            "w": weight[c * NSH : (c + 1) * NSH, :],
            "b": bias[c * NSH : (c + 1) * NSH],
        }
        for c in range(NCORES)
    ]
    res = run_bass_kernel_spmd(nc, in_maps, core_ids=list(range(NCORES)), **kw)
    out = np.concatenate([res.results[c]["out"] for c in range(NCORES)], axis=1)
    return out, res


def kernel(x: np.ndarray, weight: np.ndarray, bias: np.ndarray) -> np.ndarray:
    return _run(x, weight, bias)[0]

